# revision 5
# baseline (speedup 1.0000x reference)
"""Trainium2 Bass kernel for nn_Attention_29326036697594.

reference:
    attn = softmax(decoder @ encoder^T, axis=-1)          # [B, T, S]
    context = attn @ encoder                              # [B, T, H]
    output = tanh(concat([context, decoder]) @ W_out^T + b_out)  # [B, T, H]
    returns (output, attn)

Sharding: data-parallel over batch (B=16 over 8 cores, 2 batches/core).
Each core runs an identical single-core program (no collectives).

Per-batch device schedule (T-super = 256, i.e. 2 row-blocks of 128):
  Phase Q:  logits = D @ E^T via 3-term bf16-split matmuls (hi*hi + hi*lo +
            lo*hi; ~1e-4 absolute logit error, vs 2e-2 for fp32r / 0.34 for
            bf16, measured on HW), softmax rows on DVE(max)+ACT(exp,accum)
            +DVE(recip/scale), attn rows DMA'd out, then PE-transposes of the
            normalized attn block are staged to DRAM as A^T (fp32r).
  Phase AV+P: context^T[h,t] = sum_s E[s,h]·A^T[s,t] in fp32r, evicted to
            SBUF, then out[t,j] = tanh(ctx^T.T @ Wc^T + D^T.T @ Wd^T + 1⊗b)
            in fp32r with the bias added via a K=1 matmul, tanh on ACT.

SBUF: one 4-slot x 4MB pool rotates the three 8MB residents (E^T-split pack,
E-natural fp32, W^T fp32) across phases/batches; everything else is small.
"""
import sys
import numpy as np

sys.path.insert(0, '/opt/trn_rl_repo')

import ml_dtypes  # noqa: E402
from concourse import bass, bacc, tile, mybir  # noqa: E402
from concourse.bass_utils import run_bass_kernel_spmd  # noqa: E402
from concourse.masks import make_identity  # noqa: E402

dt = mybir.dt
P = 128

# Full problem dims
B_FULL, S_FULL, T_FULL, H_FULL = 16, 2048, 2048, 1024
N_CORES = 8


class Cfg:
    def __init__(self, B2=2, T=2048, S=2048, H=1024):
        assert T % 256 == 0 and S % 512 == 0 and H % 256 == 0
        self.B2 = B2          # batches per core
        self.T = T            # decoder length
        self.S = S            # encoder length
        self.H = H            # hidden
        self.KC = H // P      # h chunks (contraction of QK / proj halves)
        self.KCH = self.KC // 2   # kc per eT half
        self.NSUP = T // 256  # t-supers
        self.SB = S // 512    # s banks of 512 (QK free dim)
        self.SC = S // P      # s chunks of 128 (AV contraction)
        self.SCH = self.SC // 2   # sc per e half
        self.JH = H // 512    # output j halves of 512


def build(cfg: Cfg):
    """Emit the single-core SPMD program. Returns (nc, names) where names maps
    logical tensor roles to DRAM parameter names."""
    c = cfg
    nc = bacc.Bacc(None, target_bir_lowering=False, num_devices=N_CORES)

    f32, f32r, bf16 = dt.float32, dt.float32r, dt.bfloat16

    # ---- DRAM parameters (per-core shapes) ----
    # eT pack: eTp[b, half, p, kc_loc, hl, s] = split(E[b, s, (half*KCH+kc_loc)*128+p])
    eTp_d = nc.dram_tensor("eTp", [c.B2, 2, P, c.KCH, 2, c.S], bf16, kind="ExternalInput")
    # dT pack: dTp[b, p, kc, hl, t] = split(D[b, t, kc*128+p])
    dTp_d = nc.dram_tensor("dTp", [c.B2, P, c.KC, 2, c.T], bf16, kind="ExternalInput")
    # dT fp32 (read as f32r): dTr[b, p, kc, t] = D[b, t, kc*128+p]
    dTr_d = nc.dram_tensor("dTr", [c.B2, P, c.KC, c.T], f32r, kind="ExternalInput")
    # E natural halves: e[b, half, p, sc_loc, h] = E[b, (half*SCH+sc_loc)*128+p, h]
    e_d = nc.dram_tensor("e", [c.B2, 2, P, c.SCH, c.H], f32r, kind="ExternalInput")
    # W^T halves: wT[kh, p, kc_loc, j] = W_out[j, (kh*KCH... see host prep]
    wT_d = nc.dram_tensor("wT", [2, P, c.KC, c.H], f32r, kind="ExternalInput")
    bias_d = nc.dram_tensor("bias", [1, c.H], f32r, kind="ExternalInput")
    ones_d = nc.dram_tensor("ones", [1, P], f32r, kind="ExternalInput")

    attn_d = nc.dram_tensor("attn", [c.B2, c.T, c.S], f32, kind="ExternalOutput")
    out_d = nc.dram_tensor("out", [c.B2, c.T, c.H], f32, kind="ExternalOutput")

    with tile.TileContext(nc) as tc:
        with (
            tc.tile_pool(name="big", bufs=4) as big,          # 4 x 32KB/part
            tc.tile_pool(name="m1", bufs=4) as m1,            # 3 x 8KB/part (f32r 1MB tiles)
            tc.tile_pool(name="dtp", bufs=1) as dtp_pool,     # 8KB/part
            tc.tile_pool(name="apool", bufs=1) as apool,      # 2 x 8KB/part
            tc.tile_pool(name="atout", bufs=2) as atout_pool, # 4 x 512B/part
            tc.tile_pool(name="outp", bufs=1) as outp,        # 2 x 4KB/part
            tc.tile_pool(name="stats", bufs=4) as stats,
            tc.tile_pool(name="const", bufs=1) as const,
            tc.tile_pool(name="psum_mm", bufs=6, space="PSUM") as psum_mm,
            tc.tile_pool(name="psum_tp", bufs=2, space="PSUM") as psum_tp,
            tc.tile_pool(name="dram", bufs=2, space="DRAM") as dram,
        ):
            ident = const.tile([P, P], f32, tag="ident")
            make_identity(nc, ident)
            bias_sb = const.tile([1, c.H], f32r, tag="bias")
            nc.sync.dma_start(out=bias_sb[:], in_=bias_d[:])
            ones_sb = const.tile([1, P], f32r, tag="ones")
            nc.sync.dma_start(out=ones_sb[:], in_=ones_d[:])

            for b in range(c.B2):
                # ---------------- Phase Q ----------------
                # resident E^T split pack, 2 halves (kc 0..KCH-1, KCH..KC-1)
                eT = [big.tile([P, c.KCH, 2, c.S], bf16, tag="big", name=f"eT{h}") for h in range(2)]
                for h in range(2):
                    nc.sync.dma_start(out=eT[h][:], in_=eTp_d[b, h])

                at_stage = dram.tile([c.SC, P, c.T], f32r, tag="at_stage")

                for sup in range(c.NSUP):
                    t0 = sup * 256
                    dtp = dtp_pool.tile([P, c.KC, 2, 256], bf16, tag="dtp")
                    nc.sync.dma_start(out=dtp[:], in_=dTp_d[b, :, :, :, t0:t0 + 256])
                    for tsub in range(2):
                        trow = t0 + tsub * 128
                        # --- QK split matmuls into SB banks of 512 ---
                        qk = [psum_mm.tile([P, 512], f32, tag="mm512", name=f"qk{i}") for i in range(c.SB)]
                        for kc in range(c.KC):
                            eTh = eT[kc // c.KCH]
                            kcl = kc % c.KCH
                            lhs_hi = dtp[:, kc, 0, tsub * 128:tsub * 128 + 128]
                            lhs_lo = dtp[:, kc, 1, tsub * 128:tsub * 128 + 128]
                            for ti, (lh, rh) in enumerate(((0, 0), (0, 1), (1, 0))):
                                lhs = lhs_hi if lh == 0 else lhs_lo
                                first = (kc == 0 and ti == 0)
                                last = (kc == c.KC - 1 and ti == 2)
                                for sb in range(c.SB):
                                    nc.tensor.matmul(
                                        qk[sb][:],
                                        lhs,
                                        eTh[:, kcl, rh, sb * 512:sb * 512 + 512],
                                        start=first, stop=last,
                                    )
                        # --- softmax over s (free axis) ---
                        max4 = stats.tile([P, c.SB], f32, tag="max4")
                        for sb in range(c.SB):
                            nc.vector.tensor_reduce(
                                max4[:, sb:sb + 1], qk[sb][:],
                                axis=mybir.AxisListType.X, op=mybir.AluOpType.max)
                        negmax = stats.tile([P, 1], f32, tag="negmax")
                        nc.vector.tensor_reduce(
                            negmax[:], max4[:], axis=mybir.AxisListType.X,
                            op=mybir.AluOpType.max, negate=True)
                        a_sb = apool.tile([P, c.S], f32, tag="a")
                        sums = stats.tile([P, c.SB], f32, tag="sums")
                        for sb in range(c.SB):
                            nc.scalar.activation(
                                a_sb[:, sb * 512:sb * 512 + 512], qk[sb][:],
                                mybir.ActivationFunctionType.Exp,
                                bias=negmax[:], scale=1.0,
                                accum_out=sums[:, sb:sb + 1])
                        tot = stats.tile([P, 1], f32, tag="tot")
                        nc.vector.tensor_reduce(
                            tot[:], sums[:], axis=mybir.AxisListType.X,
                            op=mybir.AluOpType.add)
                        recip = stats.tile([P, 1], f32, tag="recip")
                        nc.vector.reciprocal(recip[:], tot[:])
                        nc.vector.tensor_scalar_mul(a_sb[:], a_sb[:], recip[:])
                        nc.sync.dma_start(out=attn_d[b, trow:trow + 128, :], in_=a_sb[:])
                        # --- transpose A block -> A^T staged to DRAM ---
                        for sc in range(c.SC):
                            tp = psum_tp.tile([P, P], f32, tag="tp")
                            nc.tensor.transpose(tp[:], a_sb[:, sc * P:sc * P + P], ident[:])
                            ato = atout_pool.tile([P, P], f32r, tag="ato")
                            nc.vector.tensor_copy(ato[:], tp[:])
                            nc.sync.dma_start(
                                out=at_stage[sc, :, trow:trow + 128], in_=ato[:])

                # ---------------- Phase AV + P ----------------
                e_sb = [big.tile([P, c.SCH, c.H], f32r, tag="big", name=f"e{h}") for h in range(2)]
                for h in range(2):
                    nc.sync.dma_start(out=e_sb[h][:], in_=e_d[b, h])
                wT = [big.tile([P, c.KC, c.H], f32r, tag="big", name=f"wT{h}") for h in range(2)]
                for h in range(2):
                    nc.sync.dma_start(out=wT[h][:], in_=wT_d[h])

                for sup in range(c.NSUP):
                    t0 = sup * 256
                    atin = []
                    for h in range(2):
                        ai = m1.tile([P, c.SCH, 256], f32r, tag="m1")
                        nc.sync.dma_start(
                            out=ai[:],
                            in_=at_stage[h * c.SCH:(h + 1) * c.SCH, :, t0:t0 + 256]
                                .rearrange("s p t -> p s t"))
                        atin.append(ai)
                    dtr = m1.tile([P, c.KC, 256], f32r, tag="m1")
                    nc.sync.dma_start(out=dtr[:], in_=dTr_d[b, :, :, t0:t0 + 256])

                    # context^T accumulation. An N=256 f32r matmul clobbers
                    # its full PSUM bank, so each hc needs its own bank:
                    # two half-sweeps of 4 banks (same total matmul count).
                    ctx = m1.tile([P, c.KC, 256], f32r, tag="m1")
                    for hgrp in range(2):
                        ctx_ps = [psum_mm.tile([P, 256], f32, tag="mm512",
                                               name=f"ctxps{hgrp}_{i}")
                                  for i in range(c.KC // 2)]
                        for sc in range(c.SC):
                            eh = e_sb[sc // c.SCH]
                            scl = sc % c.SCH
                            rhs = atin[sc // c.SCH][:, scl, :]
                            for hi_ in range(c.KC // 2):
                                hc = hgrp * (c.KC // 2) + hi_
                                nc.tensor.matmul(
                                    ctx_ps[hi_][:],
                                    eh[:, scl, hc * P:hc * P + P],
                                    rhs,
                                    start=(sc == 0), stop=(sc == c.SC - 1),
                                )
                        for hi_ in range(c.KC // 2):
                            hc = hgrp * (c.KC // 2) + hi_
                            nc.vector.tensor_copy(ctx[:, hc, :], ctx_ps[hi_][:])

                    # projection + bias + tanh
                    for tsub in range(2):
                        trow = t0 + tsub * 128
                        o_sb = outp.tile([P, c.H], f32, tag="o")
                        for jh in range(c.JH):
                            op = psum_mm.tile([P, 512], f32, tag="mm512")
                            for kc in range(c.KC):
                                nc.tensor.matmul(
                                    op[:],
                                    ctx[:, kc, tsub * 128:tsub * 128 + 128],
                                    wT[0][:, kc, jh * 512:jh * 512 + 512],
                                    start=(kc == 0), stop=False)
                            for kc in range(c.KC):
                                nc.tensor.matmul(
                                    op[:],
                                    dtr[:, kc, tsub * 128:tsub * 128 + 128],
                                    wT[1][:, kc, jh * 512:jh * 512 + 512],
                                    start=False, stop=False)
                            nc.tensor.matmul(
                                op[:], ones_sb[:],
                                bias_sb[:, jh * 512:jh * 512 + 512],
                                start=False, stop=True)
                            nc.scalar.activation(
                                o_sb[:, jh * 512:jh * 512 + 512], op[:],
                                mybir.ActivationFunctionType.Tanh)
                        nc.sync.dma_start(out=out_d[b, trow:trow + 128, :], in_=o_sb[:])

    nc.compile()
    return nc


def _split_bf16(x):
    hi = x.astype(ml_dtypes.bfloat16)
    lo = (x - hi.astype(np.float32)).astype(ml_dtypes.bfloat16)
    return hi, lo


def prep_core_inputs(E, D, W, bvec, cfg: Cfg):
    """Host-side layout prep for one core's batches.
    E: [B2, S, H]  D: [B2, T, H]  W: [H, 2H]  bvec: [H]  (np.float32)"""
    c = cfg
    B2 = c.B2
    # E^T pack: [B2, 2, P, KCH, 2, S]
    eT = np.ascontiguousarray(E.transpose(0, 2, 1))          # [B2, H, S]
    eT = eT.reshape(B2, 2, c.KCH, P, c.S)                     # halves x kc_loc x p
    eT = eT.transpose(0, 1, 3, 2, 4)                          # [B2, 2, P, KCH, S]
    hi, lo = _split_bf16(eT)
    eTp = np.stack([hi, lo], axis=4)                          # [B2, 2, P, KCH, 2, S]

    dT = np.ascontiguousarray(D.transpose(0, 2, 1))           # [B2, H, T]
    dTr = dT.reshape(B2, c.KC, P, c.T).transpose(0, 2, 1, 3)  # [B2, P, KC, T]
    dTr = np.ascontiguousarray(dTr)
    hi, lo = _split_bf16(dTr)
    dTp = np.stack([hi, lo], axis=3)                          # [B2, P, KC, 2, T]

    e = E.reshape(B2, 2, c.SCH, P, c.H).transpose(0, 1, 3, 2, 4)  # [B2, 2, P, SCH, H]
    e = np.ascontiguousarray(e)

    wTfull = np.ascontiguousarray(W.T)                        # [2H, H]
    # wT_d layout [2, P, KC, H]; half 0 = k<H (context part), half 1 = decoder part
    wT = wTfull.reshape(2, c.KC, P, c.H)
    wT = np.ascontiguousarray(wT.transpose(0, 2, 1, 3))

    return {
        "eTp": np.ascontiguousarray(eTp),
        "dTp": np.ascontiguousarray(dTp),
        "dTr": dTr,
        "e": e,
        "wT": wT,
        "bias": bvec.reshape(1, c.H).astype(np.float32),
        "ones": np.ones((1, P), np.float32),
    }


_NC_CACHE = {}


def _get_nc(cfg: Cfg):
    key = (cfg.B2, cfg.T, cfg.S, cfg.H)
    if key not in _NC_CACHE:
        _NC_CACHE[key] = build(cfg)
    return _NC_CACHE[key]


def run_cores(in_maps, cfg: Cfg, trace=False):
    nc = _get_nc(cfg)
    return run_bass_kernel_spmd(nc, in_maps, list(range(N_CORES)), trace=trace)


def kernel(encoder_output, decoder_output, W_out, b_out, trace=False, _res_out=None):
    """Full-problem entry: shards over batch, runs 8 cores, gathers."""
    cfg = Cfg(B2=B_FULL // N_CORES, T=T_FULL, S=S_FULL, H=H_FULL)
    E = np.asarray(encoder_output, np.float32)
    D = np.asarray(decoder_output, np.float32)
    W = np.asarray(W_out, np.float32)
    bvec = np.asarray(b_out, np.float32)

    in_maps = []
    for core in range(N_CORES):
        sl = slice(core * cfg.B2, (core + 1) * cfg.B2)
        in_maps.append(prep_core_inputs(E[sl], D[sl], W, bvec, cfg))

    res = run_cores(in_maps, cfg, trace=trace)
    if _res_out is not None:
        _res_out.append(res)

    B = E.shape[0]
    output = np.empty((B, cfg.T, cfg.H), np.float32)
    attn = np.empty((B, cfg.T, cfg.S), np.float32)
    for core in range(N_CORES):
        sl = slice(core * cfg.B2, (core + 1) * cfg.B2)
        output[sl] = res.results[core]["out"]
        attn[sl] = res.results[core]["attn"]
    return output, attn


# revision 6
# speedup vs baseline: 32.4723x; 32.4723x over previous
"""Trainium2 Bass kernel for nn_Attention_29326036697594.

reference:
    attn = softmax(decoder @ encoder^T, axis=-1)          # [B, T, S]
    context = attn @ encoder                              # [B, T, H]
    output = tanh(concat([context, decoder]) @ W_out^T + b_out)  # [B, T, H]
    returns (output, attn)

Sharding: data-parallel over batch (B=16 over 8 cores, 2 batches/core).
Each core runs an identical single-core program (no collectives).

Per-batch device schedule (T-super = 256, i.e. 2 row-blocks of 128):
  Phase Q:  logits = D @ E^T via 3-term bf16-split matmuls (hi*hi + hi*lo +
            lo*hi; ~1e-4 absolute logit error, vs 2e-2 for fp32r / 0.34 for
            bf16, measured on HW), softmax rows on DVE(max)+ACT(exp,accum)
            +DVE(recip/scale), attn rows DMA'd out, then PE-transposes of the
            normalized attn block are staged to DRAM as A^T (fp32r).
  Phase AV+P: context^T[h,t] = sum_s E[s,h]·A^T[s,t] in fp32r, evicted to
            SBUF, then out[t,j] = tanh(ctx^T.T @ Wc^T + D^T.T @ Wd^T + 1⊗b)
            in fp32r with the bias added via a K=1 matmul, tanh on ACT.

SBUF: one 4-slot x 4MB pool rotates the three 8MB residents (E^T-split pack,
E-natural fp32, W^T fp32) across phases/batches; everything else is small.
"""
import sys
import numpy as np

sys.path.insert(0, '/opt/trn_rl_repo')

import ml_dtypes  # noqa: E402
from concourse import bass, bacc, tile, mybir  # noqa: E402
from concourse.bass_utils import run_bass_kernel_spmd  # noqa: E402
from concourse.masks import make_identity  # noqa: E402

dt = mybir.dt
P = 128

# Full problem dims
B_FULL, S_FULL, T_FULL, H_FULL = 16, 2048, 2048, 1024
N_CORES = 8


class Cfg:
    def __init__(self, B2=2, T=2048, S=2048, H=1024):
        assert T % 256 == 0 and S % 512 == 0 and H % 256 == 0
        self.B2 = B2          # batches per core
        self.T = T            # decoder length
        self.S = S            # encoder length
        self.H = H            # hidden
        self.KC = H // P      # h chunks (contraction of QK / proj halves)
        self.KCH = self.KC // 2   # kc per eT half
        self.NSUP = T // 256  # t-supers
        self.SB = S // 512    # s banks of 512 (QK free dim)
        self.SC = S // P      # s chunks of 128 (AV contraction)
        self.SCH = self.SC // 2   # sc per e half
        self.JH = H // 512    # output j halves of 512


def build(cfg: Cfg, repeat=1):
    """Emit the single-core SPMD program. Returns (nc, names) where names maps
    logical tensor roles to DRAM parameter names."""
    c = cfg
    nc = bacc.Bacc(None, target_bir_lowering=False, num_devices=N_CORES)

    f32, f32r, bf16 = dt.float32, dt.float32r, dt.bfloat16

    # ---- DRAM parameters (per-core shapes) ----
    # eT pack: eTp[b, half, p, kc_loc, hl, s] = split(E[b, s, (half*KCH+kc_loc)*128+p])
    eTp_d = nc.dram_tensor("eTp", [c.B2, 2, P, c.KCH, 2, c.S], bf16, kind="ExternalInput")
    # dT pack: dTp[b, p, kc, hl, t] = split(D[b, t, kc*128+p])
    dTp_d = nc.dram_tensor("dTp", [c.B2, P, c.KC, 2, c.T], bf16, kind="ExternalInput")
    # dT fp32 (read as f32r): dTr[b, p, kc, t] = D[b, t, kc*128+p]
    dTr_d = nc.dram_tensor("dTr", [c.B2, P, c.KC, c.T], f32r, kind="ExternalInput")
    # E natural halves: e[b, half, p, sc_loc, h] = E[b, (half*SCH+sc_loc)*128+p, h]
    e_d = nc.dram_tensor("e", [c.B2, 2, P, c.SCH, c.H], f32r, kind="ExternalInput")
    # W^T halves: wT[kh, p, kc_loc, j] = W_out[j, (kh*KCH... see host prep]
    wT_d = nc.dram_tensor("wT", [2, P, c.KC, c.H], f32r, kind="ExternalInput")
    bias_d = nc.dram_tensor("bias", [1, c.H], f32r, kind="ExternalInput")
    ones_d = nc.dram_tensor("ones", [1, P], f32r, kind="ExternalInput")

    attn_d = nc.dram_tensor("attn", [c.B2, c.T, c.S], f32, kind="ExternalOutput")
    out_d = nc.dram_tensor("out", [c.B2, c.T, c.H], f32, kind="ExternalOutput")

    with tile.TileContext(nc) as tc:
        with (
            tc.tile_pool(name="big", bufs=4) as big,          # 4 x 32KB/part
            tc.tile_pool(name="m1", bufs=4) as m1,            # 3 x 8KB/part (f32r 1MB tiles)
            tc.tile_pool(name="dtp", bufs=1) as dtp_pool,     # 8KB/part
            tc.tile_pool(name="apool", bufs=1) as apool,      # 2 x 8KB/part
            tc.tile_pool(name="atout", bufs=2) as atout_pool, # 4 x 512B/part
            tc.tile_pool(name="outp", bufs=1) as outp,        # 2 x 4KB/part
            tc.tile_pool(name="stats", bufs=4) as stats,
            tc.tile_pool(name="const", bufs=1) as const,
            tc.tile_pool(name="psum_mm", bufs=6, space="PSUM") as psum_mm,
            tc.tile_pool(name="psum_tp", bufs=2, space="PSUM") as psum_tp,
            tc.tile_pool(name="dram", bufs=2, space="DRAM") as dram,
        ):
            ident = const.tile([P, P], f32, tag="ident")
            make_identity(nc, ident)
            bias_sb = const.tile([1, c.H], f32r, tag="bias")
            nc.sync.dma_start(out=bias_sb[:], in_=bias_d[:])
            ones_sb = const.tile([1, P], f32r, tag="ones")
            nc.sync.dma_start(out=ones_sb[:], in_=ones_d[:])

            for b in [bb for _ in range(repeat) for bb in range(c.B2)]:
                # ---------------- Phase Q ----------------
                # resident E^T split pack, 2 halves (kc 0..KCH-1, KCH..KC-1)
                eT = [big.tile([P, c.KCH, 2, c.S], bf16, tag="big", name=f"eT{h}") for h in range(2)]
                for h in range(2):
                    nc.sync.dma_start(out=eT[h][:], in_=eTp_d[b, h])

                at_stage = dram.tile([c.SC, P, c.T], f32r, tag="at_stage")

                for sup in range(c.NSUP):
                    t0 = sup * 256
                    dtp = dtp_pool.tile([P, c.KC, 2, 256], bf16, tag="dtp")
                    nc.sync.dma_start(out=dtp[:], in_=dTp_d[b, :, :, :, t0:t0 + 256])
                    for tsub in range(2):
                        trow = t0 + tsub * 128
                        # --- QK split matmuls into SB banks of 512 ---
                        qk = [psum_mm.tile([P, 512], f32, tag="mm512", name=f"qk{i}") for i in range(c.SB)]
                        for kc in range(c.KC):
                            eTh = eT[kc // c.KCH]
                            kcl = kc % c.KCH
                            lhs_hi = dtp[:, kc, 0, tsub * 128:tsub * 128 + 128]
                            lhs_lo = dtp[:, kc, 1, tsub * 128:tsub * 128 + 128]
                            for ti, (lh, rh) in enumerate(((0, 0), (0, 1), (1, 0))):
                                lhs = lhs_hi if lh == 0 else lhs_lo
                                first = (kc == 0 and ti == 0)
                                last = (kc == c.KC - 1 and ti == 2)
                                for sb in range(c.SB):
                                    nc.tensor.matmul(
                                        qk[sb][:],
                                        lhs,
                                        eTh[:, kcl, rh, sb * 512:sb * 512 + 512],
                                        start=first, stop=last,
                                    )
                        # --- softmax over s (free axis) ---
                        max4 = stats.tile([P, c.SB], f32, tag="max4")
                        for sb in range(c.SB):
                            nc.vector.tensor_reduce(
                                max4[:, sb:sb + 1], qk[sb][:],
                                axis=mybir.AxisListType.X, op=mybir.AluOpType.max)
                        negmax = stats.tile([P, 1], f32, tag="negmax")
                        nc.vector.tensor_reduce(
                            negmax[:], max4[:], axis=mybir.AxisListType.X,
                            op=mybir.AluOpType.max, negate=True)
                        a_sb = apool.tile([P, c.S], f32, tag="a")
                        sums = stats.tile([P, c.SB], f32, tag="sums")
                        for sb in range(c.SB):
                            nc.scalar.activation(
                                a_sb[:, sb * 512:sb * 512 + 512], qk[sb][:],
                                mybir.ActivationFunctionType.Exp,
                                bias=negmax[:], scale=1.0,
                                accum_out=sums[:, sb:sb + 1])
                        tot = stats.tile([P, 1], f32, tag="tot")
                        nc.vector.tensor_reduce(
                            tot[:], sums[:], axis=mybir.AxisListType.X,
                            op=mybir.AluOpType.add)
                        recip = stats.tile([P, 1], f32, tag="recip")
                        nc.vector.reciprocal(recip[:], tot[:])
                        nc.vector.tensor_scalar_mul(a_sb[:], a_sb[:], recip[:])
                        nc.sync.dma_start(out=attn_d[b, trow:trow + 128, :], in_=a_sb[:])
                        # --- transpose A block -> A^T staged to DRAM ---
                        for sc in range(c.SC):
                            tp = psum_tp.tile([P, P], f32, tag="tp")
                            nc.tensor.transpose(tp[:], a_sb[:, sc * P:sc * P + P], ident[:])
                            ato = atout_pool.tile([P, P], f32r, tag="ato")
                            nc.vector.tensor_copy(ato[:], tp[:])
                            nc.sync.dma_start(
                                out=at_stage[sc, :, trow:trow + 128], in_=ato[:])

                # ---------------- Phase AV + P ----------------
                e_sb = [big.tile([P, c.SCH, c.H], f32r, tag="big", name=f"e{h}") for h in range(2)]
                for h in range(2):
                    nc.sync.dma_start(out=e_sb[h][:], in_=e_d[b, h])
                wT = [big.tile([P, c.KC, c.H], f32r, tag="big", name=f"wT{h}") for h in range(2)]
                for h in range(2):
                    nc.sync.dma_start(out=wT[h][:], in_=wT_d[h])

                for sup in range(c.NSUP):
                    t0 = sup * 256
                    atin = []
                    for h in range(2):
                        ai = m1.tile([P, c.SCH, 256], f32r, tag="m1")
                        nc.sync.dma_start(
                            out=ai[:],
                            in_=at_stage[h * c.SCH:(h + 1) * c.SCH, :, t0:t0 + 256]
                                .rearrange("s p t -> p s t"))
                        atin.append(ai)
                    dtr = m1.tile([P, c.KC, 256], f32r, tag="m1")
                    nc.sync.dma_start(out=dtr[:], in_=dTr_d[b, :, :, t0:t0 + 256])

                    # context^T accumulation. An N=256 f32r matmul clobbers
                    # its full PSUM bank, so each hc needs its own bank:
                    # two half-sweeps of 4 banks (same total matmul count).
                    ctx = m1.tile([P, c.KC, 256], f32r, tag="m1")
                    for hgrp in range(2):
                        ctx_ps = [psum_mm.tile([P, 256], f32, tag="mm512",
                                               name=f"ctxps{hgrp}_{i}")
                                  for i in range(c.KC // 2)]
                        for sc in range(c.SC):
                            eh = e_sb[sc // c.SCH]
                            scl = sc % c.SCH
                            rhs = atin[sc // c.SCH][:, scl, :]
                            for hi_ in range(c.KC // 2):
                                hc = hgrp * (c.KC // 2) + hi_
                                nc.tensor.matmul(
                                    ctx_ps[hi_][:],
                                    eh[:, scl, hc * P:hc * P + P],
                                    rhs,
                                    start=(sc == 0), stop=(sc == c.SC - 1),
                                )
                        for hi_ in range(c.KC // 2):
                            hc = hgrp * (c.KC // 2) + hi_
                            nc.vector.tensor_copy(ctx[:, hc, :], ctx_ps[hi_][:])

                    # projection + bias + tanh
                    for tsub in range(2):
                        trow = t0 + tsub * 128
                        o_sb = outp.tile([P, c.H], f32, tag="o")
                        for jh in range(c.JH):
                            op = psum_mm.tile([P, 512], f32, tag="mm512")
                            for kc in range(c.KC):
                                nc.tensor.matmul(
                                    op[:],
                                    ctx[:, kc, tsub * 128:tsub * 128 + 128],
                                    wT[0][:, kc, jh * 512:jh * 512 + 512],
                                    start=(kc == 0), stop=False)
                            for kc in range(c.KC):
                                nc.tensor.matmul(
                                    op[:],
                                    dtr[:, kc, tsub * 128:tsub * 128 + 128],
                                    wT[1][:, kc, jh * 512:jh * 512 + 512],
                                    start=False, stop=False)
                            nc.tensor.matmul(
                                op[:], ones_sb[:],
                                bias_sb[:, jh * 512:jh * 512 + 512],
                                start=False, stop=True)
                            nc.scalar.activation(
                                o_sb[:, jh * 512:jh * 512 + 512], op[:],
                                mybir.ActivationFunctionType.Tanh)
                        nc.sync.dma_start(out=out_d[b, trow:trow + 128, :], in_=o_sb[:])

    nc.compile()
    return nc


def _split_bf16(x):
    hi = x.astype(ml_dtypes.bfloat16)
    lo = (x - hi.astype(np.float32)).astype(ml_dtypes.bfloat16)
    return hi, lo


def prep_core_inputs(E, D, W, bvec, cfg: Cfg):
    """Host-side layout prep for one core's batches.
    E: [B2, S, H]  D: [B2, T, H]  W: [H, 2H]  bvec: [H]  (np.float32)"""
    c = cfg
    B2 = c.B2
    # E^T pack: [B2, 2, P, KCH, 2, S]
    eT = np.ascontiguousarray(E.transpose(0, 2, 1))          # [B2, H, S]
    eT = eT.reshape(B2, 2, c.KCH, P, c.S)                     # halves x kc_loc x p
    eT = eT.transpose(0, 1, 3, 2, 4)                          # [B2, 2, P, KCH, S]
    hi, lo = _split_bf16(eT)
    eTp = np.stack([hi, lo], axis=4)                          # [B2, 2, P, KCH, 2, S]

    dT = np.ascontiguousarray(D.transpose(0, 2, 1))           # [B2, H, T]
    dTr = dT.reshape(B2, c.KC, P, c.T).transpose(0, 2, 1, 3)  # [B2, P, KC, T]
    dTr = np.ascontiguousarray(dTr)
    hi, lo = _split_bf16(dTr)
    dTp = np.stack([hi, lo], axis=3)                          # [B2, P, KC, 2, T]

    e = E.reshape(B2, 2, c.SCH, P, c.H).transpose(0, 1, 3, 2, 4)  # [B2, 2, P, SCH, H]
    e = np.ascontiguousarray(e)

    wTfull = np.ascontiguousarray(W.T)                        # [2H, H]
    # wT_d layout [2, P, KC, H]; half 0 = k<H (context part), half 1 = decoder part
    wT = wTfull.reshape(2, c.KC, P, c.H)
    wT = np.ascontiguousarray(wT.transpose(0, 2, 1, 3))

    return {
        "eTp": np.ascontiguousarray(eTp),
        "dTp": np.ascontiguousarray(dTp),
        "dTr": dTr,
        "e": e,
        "wT": wT,
        "bias": bvec.reshape(1, c.H).astype(np.float32),
        "ones": np.ones((1, P), np.float32),
    }


_NC_CACHE = {}


def _get_nc(cfg: Cfg, repeat=1):
    key = (cfg.B2, cfg.T, cfg.S, cfg.H, repeat)
    if key not in _NC_CACHE:
        _NC_CACHE[key] = build(cfg, repeat=repeat)
    return _NC_CACHE[key]


def run_cores(in_maps, cfg: Cfg, trace=False):
    nc = _get_nc(cfg)
    return run_bass_kernel_spmd(nc, in_maps, list(range(N_CORES)), trace=trace)


def kernel(encoder_output, decoder_output, W_out, b_out, trace=False, _res_out=None):
    """Full-problem entry: shards over batch, runs 8 cores, gathers."""
    cfg = Cfg(B2=B_FULL // N_CORES, T=T_FULL, S=S_FULL, H=H_FULL)
    E = np.asarray(encoder_output, np.float32)
    D = np.asarray(decoder_output, np.float32)
    W = np.asarray(W_out, np.float32)
    bvec = np.asarray(b_out, np.float32)

    in_maps = []
    for core in range(N_CORES):
        sl = slice(core * cfg.B2, (core + 1) * cfg.B2)
        in_maps.append(prep_core_inputs(E[sl], D[sl], W, bvec, cfg))

    res = run_cores(in_maps, cfg, trace=trace)
    if _res_out is not None:
        _res_out.append(res)

    B = E.shape[0]
    output = np.empty((B, cfg.T, cfg.H), np.float32)
    attn = np.empty((B, cfg.T, cfg.S), np.float32)
    for core in range(N_CORES):
        sl = slice(core * cfg.B2, (core + 1) * cfg.B2)
        output[sl] = res.results[core]["out"]
        attn[sl] = res.results[core]["attn"]
    return output, attn
